# revision 18
# baseline (speedup 1.0000x reference)
"""CrossAttention kernel for Trainium2 (8 NeuronCores).

Problem: B=4, Sq=Sk=2048, H=16 heads, D=64, NUM_HIDDEN=1024.
query/key are (B, S, 1) and Wq/Wk are (1, 1024) -- the q/k projections are
rank-1.  The attention logits therefore factor as

  logits[i,j] = (q_i . k_j)/8 = A_h x_i y_j/8 + C_h x_i/8 + E_h y_j/8 + F_h/8

with per-head scalars A_h = Wq_h.Wk_h, E_h = Wk_h.bq_h (x = query[...,0],
y = key[...,0]).  Terms constant in j cancel under softmax over j, so

  attn[i, :] = softmax_j( scale_j * x_i + bias_j ),
  scale_j = A_h y_j / 8,   bias_j = E_h y_j / 8.

On device (per core: one batch b, 8 heads, everything bf16 on the PE):
  1. V projection: V = value_b @ Wv[:, headcols] + bv   (PE matmul, K=1024)
  2. T[j, i] = exp(scale_j * x_i + bias_j) -- ONE ScalarE activation per
     (head, j-tile): exp with per-partition scale/bias, free dim = 2048,
     bf16 output.  The ScalarE exp stream is the bottleneck engine; a deep
     tte ring buffer keeps it running back-to-back from t=0.
  3. numerator/denominator in one PE matmul: lhsT = [V_h | 1] (j x 65),
     rhs = T (j x 512 chunks), accumulated over 16 j-tiles in PSUM.
  4. PE-transpose [65, 128] chunks -> [128, 65], reciprocal of the Z
     column on [128, 4] per-partition data, tensor_scalar_mul on GpSimd,
     per-head SBUF-accumulated output DMA with 4KB partition lines.

Sharding: core c -> batch b = c // 2, head group g = c % 2 (8 heads each).
"""

import sys

import numpy as np

for _p in ("/opt/trn_rl_repo",):
    if _p not in sys.path:
        sys.path.insert(0, _p)

B = 4
S = 2048
H = 16
D = 64
NH = 1024
P = 128
JT = S // P          # 16 j-tiles
HPC = 8              # heads per core
HT = NH // P         # 8 hidden tiles
IC = 4               # i-chunks of 512
ICW = 512
N_CORES = 8
TBUFS = 16           # tte ring depth: covers ScalarE lead over phase-1 VP

# --- DVE/Pool exp offload (Schraudolph + quadratic mantissa fix) ---
# Quadratic lsq fit of 2^(m-1)/m on [1,2): c(m) = A2 m^2 + A1 m + A0.
# The global 1/A0 factor is folded into every tile (softmax-invariant):
# ScalarE tiles get bias - ln(A0); DVE tiles produce exp/A0 natively.
EXPL = float(2 ** 23)
LOG2E = 1.4426950408889634
A2F, A1F, A0F = 0.22574, -0.66667151, 1.43449126
B2F = A2F / A0F
B1F = A1F / A0F
import numpy as _np_for_mask
MASKF = float(_np_for_mask.int32(0x007FFFFF).view(_np_for_mask.float32))
# per-head offload plan: jt -> engine for the affine+round step
OFF_JTS = {2: "pool", 4: "pool", 7: "pool", 9: "pool", 12: "pool", 14: "pool"}

_cache = {}


def _register_exp_op():
    from concourse import dve_ops as dmod
    from concourse.dve_spec import (
        C0,
        C1,
        C2,
        AluOp,
        Bin,
        One,
        Spec,
        Src0,
        Src1,
        lower,
    )
    from concourse.dve_spec import _has_src1 as has_src1
    from concourse.dve_uop import DveOpSpec

    name = "EXP_SCHRAUD_ANT"
    for o in dmod.OPS:
        if o.name == name:
            return o
    m = Bin(AluOp.BITWISE_OR, Bin(AluOp.BITWISE_AND, Src0, C0), C1)
    body = ((m * C2 + Src1) * m + One) * Src0

    def _ref(in0, in1, s0, s1, imm2):
        bits = np.asarray(in0, np.float32).view(np.int32)
        mk = np.float32(s0).view(np.int32)
        eb = np.float32(s1).view(np.int32)
        mm = ((bits & mk) | eb).view(np.float32)
        return ((mm * np.float32(imm2) + in1) * mm + np.float32(1.0)) * np.asarray(
            in0, np.float32
        )

    spec = Spec(body=body, reference=_ref)
    row = dmod._CUSTOM_DVE_ROW_BASE + len(dmod.OPS)
    shas = {}
    for ver in ("v3", "v4"):
        u = lower(spec, ver=ver)
        shas[ver] = DveOpSpec(
            name=name, opcode=row, uops=u, rd1_en=has_src1(spec)
        ).sha(ver)
    op = dmod.DveOp(name, spec, subdim=False, uops_sha=shas)
    dmod.OPS.append(op)
    dmod.CUSTOM_DVE_SPECS[name] = spec
    dmod._SUB_OPCODE_FOR_NAME[name] = row
    return op


def _build_program():
    import concourse.bass as bass  # noqa: F401
    import concourse.mybir as mybir
    from concourse import bacc
    from concourse.masks import make_identity
    from concourse.tile import TileContext

    f32 = mybir.dt.float32
    bf16 = mybir.dt.bfloat16
    i32 = mybir.dt.int32

    expop = _register_exp_op()

    nc = bacc.Bacc(trn_type="TRN2")

    # [p, ht, j]: partition line = HT*S*2B = 32 KB contiguous
    valueT = nc.dram_tensor("valueT", [P, HT, S], bf16, kind="ExternalInput")
    # [p, ht, col]: partition line = HT*512*2B = 8 KB contiguous
    wv = nc.dram_tensor("wv", [P, HT, HPC * D], bf16, kind="ExternalInput")
    bvs = nc.dram_tensor("bvs", [1, HPC * D], bf16, kind="ExternalInput")
    # meta: per-partition [sb | eb-ln(A0) | ss | bb], each JT*HPC wide
    meta = nc.dram_tensor("meta", [P, 4 * JT * HPC], f32, kind="ExternalInput")
    # x broadcast to all partitions, fp16 (2 elem/cycle on ScalarE)
    xh = nc.dram_tensor("xh", [P, S], f32, kind="ExternalInput")
    # [hl, p, icq, d]: row i = icq*128 + p; partition line = JT*D*4B = 4 KB
    out = nc.dram_tensor("out", [HPC, P, JT, D], f32, kind="ExternalOutput")

    with TileContext(nc) as tc:
        with (
            tc.tile_pool(name="const", bufs=1) as const_pool,
            tc.tile_pool(name="vp", bufs=1) as vp_pool,
            tc.tile_pool(name="tt", bufs=TBUFS) as t_pool,
            tc.tile_pool(name="zz", bufs=3) as z_pool,
            tc.tile_pool(name="ps", bufs=2, space="PSUM") as ps_pool,
            tc.tile_pool(name="av", bufs=4, space="PSUM") as av_pool,
            tc.tile_pool(name="tp", bufs=2, space="PSUM") as tp_pool,
            tc.tile_pool(name="sp", bufs=4) as s_pool,
            tc.tile_pool(name="cp", bufs=3) as c_pool,
            tc.tile_pool(name="rp", bufs=3) as r_pool,
            tc.tile_pool(name="op", bufs=2) as o_pool,
        ):
            ident = const_pool.tile([P, P], f32)
            make_identity(nc, ident)
            ones1 = const_pool.tile([1, P], bf16)
            nc.vector.memset(ones1[:, :], 1.0)
            # meta first: the ScalarE exp stream (the bottleneck engine)
            # starts as soon as it lands; valueT is only needed ~15us in.
            meta_sb = const_pool.tile([P, 4 * JT * HPC], f32)
            nc.sync.dma_start(meta_sb[:, :], meta[:, :])
            xh_sb = const_pool.tile([P, S], f32)
            nc.sync.dma_start(xh_sb[:, :], xh[:, :])
            bv_sb = const_pool.tile([1, HPC * D], bf16)
            nc.sync.dma_start(bv_sb[:, :], bvs[:, :])
            wv_sb = const_pool.tile([P, HT, HPC * D], bf16)
            nc.sync.dma_start(wv_sb[:, :, :], wv[:, :, :])
            vt_sb = const_pool.tile([P, HT, S], bf16)
            for jc in range(4):
                nc.sync.dma_start(
                    vt_sb[:, :, jc * (S // 4) : (jc + 1) * (S // 4)],
                    valueT[:, :, jc * (S // 4) : (jc + 1) * (S // 4)],
                )
            sb_sb = meta_sb[:, 0 : JT * HPC].rearrange(
                "p (jt h) -> p jt h", h=HPC
            )
            eb_sb = meta_sb[:, JT * HPC : 2 * JT * HPC].rearrange(
                "p (jt h) -> p jt h", h=HPC
            )
            ss_sb = meta_sb[:, 2 * JT * HPC : 3 * JT * HPC].rearrange(
                "p (jt h) -> p jt h", h=HPC
            )
            bb_sb = meta_sb[:, 3 * JT * HPC : 4 * JT * HPC].rearrange(
                "p (jt h) -> p jt h", h=HPC
            )
            b1sb = const_pool.tile([P, S], f32)
            nc.gpsimd.memset(b1sb[:, :], B1F)

            # V-plus: per head, [j-part, jt, D+1]; column D preset to 1.0 so
            # the AV matmul also produces the softmax denominator (row D).
            vp = vp_pool.tile([P, HPC, JT, D + 1], bf16)
            nc.gpsimd.memset(vp[:, :, :, D : D + 1], 1.0)

            # Phase 1 VP is interleaved with head 0's phase 2 below, so
            # the PE starts consuming tte tiles immediately and the exp
            # engines never stall on t_pool backpressure.
            def emit_vp_jt(jt):
                ps = ps_pool.tile([P, HPC * D], f32, space="PSUM")
                for ht in range(HT):
                    nc.tensor.matmul(
                        ps,
                        vt_sb[:, ht, jt * P : (jt + 1) * P],
                        wv_sb[:, ht, :],
                        start=(ht == 0),
                        stop=False,
                    )
                nc.tensor.matmul(
                    ps, ones1[:, :], bv_sb[:, :], start=False, stop=True
                )
                nc.vector.tensor_copy(
                    vp[:, :, jt, 0:D],
                    ps.rearrange("p (h d) -> p h d", h=HPC),
                )

            def emit_tte(hl, jt):
                tte = t_pool.tile([P, S], bf16)
                eng = OFF_JTS.get(jt)
                if eng is None:
                    nc.scalar.activation(
                        tte,
                        xh_sb,
                        mybir.ActivationFunctionType.Exp,
                        bias=eb_sb[:, jt, hl : hl + 1],
                        scale=sb_sb[:, jt, hl : hl + 1],
                    )
                else:
                    z = z_pool.tile([P, S], i32)
                    tsc = nc.gpsimd if eng == "pool" else nc.vector
                    tsc.tensor_scalar(
                        z,
                        xh_sb,
                        ss_sb[:, jt, hl : hl + 1],
                        bb_sb[:, jt, hl : hl + 1],
                        mybir.AluOpType.mult,
                        mybir.AluOpType.add,
                    )
                    nc.vector._custom_dve(
                        expop,
                        out=tte,
                        in0=z.bitcast(f32),
                        in1=b1sb,
                        s0=MASKF,
                        s1=1.0,
                        imm2=B2F,
                    )
                return tte

            def alloc_avs():
                return [
                    av_pool.tile(
                        [D + 1, ICW], f32, name=f"av{ic}", tag="av", space="PSUM"
                    )
                    for ic in range(IC)
                ]

            def emit_av(avs, hl, jt, tte):
                for ic in range(IC):
                    nc.tensor.matmul(
                        avs[ic],
                        vp[:, hl, jt, :],
                        tte[:, ic * ICW : (ic + 1) * ICW],
                        start=(jt == 0),
                        stop=(jt == JT - 1),
                    )

            def emit_drain(hl, avs):
                otile = o_pool.tile([P, IC * IC, D], f32)
                stens = []
                for ic in range(IC):
                    sten = s_pool.tile([D + 1, ICW], f32)
                    nc.vector.tensor_copy(sten, avs[ic])
                    stens.append(sten)
                for ic in range(IC):
                    sten = stens[ic]
                    tp = tp_pool.tile([P, IC, D + 1], f32, space="PSUM")
                    for q in range(IC):
                        nc.tensor.transpose(
                            tp[:, q, :],
                            sten[:, q * P : (q + 1) * P],
                            ident[0 : D + 1, 0 : D + 1],
                        )
                    ctile = c_pool.tile([P, IC, D + 1], f32)
                    nc.vector.tensor_copy(ctile, tp)
                    rec = r_pool.tile([P, IC, 1], f32)
                    nc.vector.reciprocal(rec, ctile[:, :, D : D + 1])
                    for q in range(IC):
                        nc.vector.tensor_scalar_mul(
                            otile[:, ic * IC + q, :],
                            ctile[:, q, 0:D],
                            rec[:, q, :],
                        )
                    nc.sync.dma_start(
                        out[hl, :, ic * IC : (ic + 1) * IC, :],
                        otile[:, ic * IC : (ic + 1) * IC, :],
                    )

            # head 0 rides along with the V projection (one-jt software
            # pipeline so its AV matmuls never block the PE on the cast)
            avs0 = alloc_avs()
            emit_vp_jt(0)
            for jt in range(JT):
                if jt + 1 < JT:
                    emit_vp_jt(jt + 1)
                tte = emit_tte(0, jt)
                emit_av(avs0, 0, jt, tte)
            emit_drain(0, avs0)

            for hl in range(1, HPC):
                avs = alloc_avs()
                for jt in range(JT):
                    tte = emit_tte(hl, jt)
                    emit_av(avs, hl, jt, tte)
                emit_drain(hl, avs)
    nc.compile()  # bacc legalization: wait-splitting, reg alloc, nop fusion
    return nc


def _get_program():
    if "nc" not in _cache:
        _cache["nc"] = _build_program()
    return _cache["nc"]


def kernel(query, key, value, Wq, bq, Wk, bk, Wv, bv):
    import concourse.mybir as mybir
    from concourse import bass_utils

    bf16 = mybir.dt.np(mybir.dt.bfloat16)

    query = np.asarray(query, dtype=np.float32)
    key = np.asarray(key, dtype=np.float32)
    value = np.asarray(value, dtype=np.float32)
    Wq = np.asarray(Wq, dtype=np.float32)
    bq = np.asarray(bq, dtype=np.float32)
    Wk = np.asarray(Wk, dtype=np.float32)
    bk = np.asarray(bk, dtype=np.float32)
    Wv = np.asarray(Wv, dtype=np.float32)
    bv = np.asarray(bv, dtype=np.float32)

    wq2 = Wq.reshape(H, D)
    wk2 = Wk.reshape(H, D)
    bq2 = bq.reshape(H, D)
    A = np.einsum("hd,hd->h", wq2, wk2)  # Wq_h . Wk_h
    E = np.einsum("hd,hd->h", wk2, bq2)  # Wk_h . bq_h

    in_maps = []
    for c in range(N_CORES):
        b = c // 2
        g = c % 2
        heads = np.arange(g * HPC, (g + 1) * HPC)
        x = query[b, :, 0]  # (S,)
        y = key[b, :, 0]  # (S,)
        # scale[j, h] = A_h y_j / 8 ; bias[j, h] = E_h y_j / 8
        sc = (y[:, None] * (A[heads] / 8.0)[None, :]).astype(np.float32)
        bi = (y[:, None] * (E[heads] / 8.0)[None, :]).astype(np.float32)
        amax = np.abs(sc * np.abs(x).max() + np.abs(bi)).max()
        assert amax < 80.0, f"logit magnitude {amax} risks fp32 exp overflow"
        in_maps.append(
            {
                # [p, ht, j] = value[b][j, ht*128+p]
                "valueT": np.ascontiguousarray(
                    value[b].T.reshape(HT, P, S).transpose(1, 0, 2)
                ).astype(bf16),
                # [p, ht, col] = Wv[ht*128+p, g*512+col]
                "wv": np.ascontiguousarray(
                    Wv[:, g * HPC * D : (g + 1) * HPC * D]
                    .reshape(HT, P, HPC * D)
                    .transpose(1, 0, 2)
                ).astype(bf16),
                "bvs": bv[g * HPC * D : (g + 1) * HPC * D].reshape(1, HPC * D).astype(bf16),
                "meta": np.concatenate(
                    [
                        sc.reshape(JT, P, HPC).transpose(1, 0, 2).reshape(P, -1),
                        (bi - np.log(A0F))
                        .reshape(JT, P, HPC)
                        .transpose(1, 0, 2)
                        .reshape(P, -1),
                        (sc * LOG2E * EXPL)
                        .reshape(JT, P, HPC)
                        .transpose(1, 0, 2)
                        .reshape(P, -1),
                        (bi * LOG2E * EXPL + 127.0 * EXPL)
                        .reshape(JT, P, HPC)
                        .transpose(1, 0, 2)
                        .reshape(P, -1),
                    ],
                    axis=1,
                ).astype(np.float32),
                "xh": np.ascontiguousarray(
                    np.broadcast_to(x, (P, S))
                ).astype(np.float32),
            }
        )

    nc = _get_program()
    res = bass_utils.run_bass_kernel_spmd(
        nc, in_maps, core_ids=list(range(N_CORES))
    ).results

    full = np.empty((H * B, S, D), dtype=np.float32)
    for c in range(N_CORES):
        b = c // 2
        g = c % 2
        o = res[c]["out"]  # [HPC, P, JT, D]; row i = icq*128 + p
        for hl in range(HPC):
            full[(g * HPC + hl) * B + b] = (
                o[hl].transpose(1, 0, 2).reshape(S, D)
            )
    return full


# revision 19
# speedup vs baseline: 1.0530x; 1.0530x over previous
"""CrossAttention kernel for Trainium2 (8 NeuronCores).

Problem: B=4, Sq=Sk=2048, H=16 heads, D=64, NUM_HIDDEN=1024.
query/key are (B, S, 1) and Wq/Wk are (1, 1024) -- the q/k projections are
rank-1.  The attention logits therefore factor as

  logits[i,j] = (q_i . k_j)/8 = A_h x_i y_j/8 + C_h x_i/8 + E_h y_j/8 + F_h/8

with per-head scalars A_h = Wq_h.Wk_h, E_h = Wk_h.bq_h (x = query[...,0],
y = key[...,0]).  Terms constant in j cancel under softmax over j, so

  attn[i, :] = softmax_j( scale_j * x_i + bias_j ),
  scale_j = A_h y_j / 8,   bias_j = E_h y_j / 8.

On device (per core: one batch b, 8 heads, everything bf16 on the PE):
  1. V projection: V = value_b @ Wv[:, headcols] + bv   (PE matmul, K=1024)
  2. T[j, i] = exp(scale_j * x_i + bias_j) -- ONE ScalarE activation per
     (head, j-tile): exp with per-partition scale/bias, free dim = 2048,
     bf16 output.  The ScalarE exp stream is the bottleneck engine; a deep
     tte ring buffer keeps it running back-to-back from t=0.
  3. numerator/denominator in one PE matmul: lhsT = [V_h | 1] (j x 65),
     rhs = T (j x 512 chunks), accumulated over 16 j-tiles in PSUM.
  4. PE-transpose [65, 128] chunks -> [128, 65], reciprocal of the Z
     column on [128, 4] per-partition data, tensor_scalar_mul on GpSimd,
     per-head SBUF-accumulated output DMA with 4KB partition lines.

Sharding: core c -> batch b = c // 2, head group g = c % 2 (8 heads each).
"""

import sys

import numpy as np

for _p in ("/opt/trn_rl_repo",):
    if _p not in sys.path:
        sys.path.insert(0, _p)

B = 4
S = 2048
H = 16
D = 64
NH = 1024
P = 128
JT = S // P          # 16 j-tiles
HPC = 8              # heads per core
HT = NH // P         # 8 hidden tiles
IC = 4               # i-chunks of 512
ICW = 512
N_CORES = 8
TBUFS = 18           # tte ring depth: covers ScalarE lead over phase-1 VP

# --- DVE/Pool exp offload (Schraudolph + quadratic mantissa fix) ---
# Quadratic lsq fit of 2^(m-1)/m on [1,2): c(m) = A2 m^2 + A1 m + A0.
# The global 1/A0 factor is folded into every tile (softmax-invariant):
# ScalarE tiles get bias - ln(A0); DVE tiles produce exp/A0 natively.
EXPL = float(2 ** 23)
LOG2E = 1.4426950408889634
A2F, A1F, A0F = 0.22574, -0.66667151, 1.43449126
B2F = A2F / A0F
B1F = A1F / A0F
import numpy as _np_for_mask
MASKF = float(_np_for_mask.int32(0x007FFFFF).view(_np_for_mask.float32))
# per-head offload plan: jt -> engine for the affine+round step
BASE_OFF = (2, 4, 7, 9, 12, 14)  # offloaded j-tiles (affine+round on Pool)


def off_jts_for(hl):
    # heads 0-1 run one fewer offload: the DVE also does the phase-1
    # V-projection casts during that window
    return BASE_OFF[:2] + BASE_OFF[3:] if hl < 2 else BASE_OFF

_cache = {}


def _register_exp_op():
    from concourse import dve_ops as dmod
    from concourse.dve_spec import (
        C0,
        C1,
        C2,
        AluOp,
        Bin,
        One,
        Spec,
        Src0,
        Src1,
        lower,
    )
    from concourse.dve_spec import _has_src1 as has_src1
    from concourse.dve_uop import DveOpSpec

    name = "EXP_SCHRAUD_ANT"
    for o in dmod.OPS:
        if o.name == name:
            return o
    m = Bin(AluOp.BITWISE_OR, Bin(AluOp.BITWISE_AND, Src0, C0), C1)
    body = ((m * C2 + Src1) * m + One) * Src0

    def _ref(in0, in1, s0, s1, imm2):
        bits = np.asarray(in0, np.float32).view(np.int32)
        mk = np.float32(s0).view(np.int32)
        eb = np.float32(s1).view(np.int32)
        mm = ((bits & mk) | eb).view(np.float32)
        return ((mm * np.float32(imm2) + in1) * mm + np.float32(1.0)) * np.asarray(
            in0, np.float32
        )

    spec = Spec(body=body, reference=_ref)
    row = dmod._CUSTOM_DVE_ROW_BASE + len(dmod.OPS)
    shas = {}
    for ver in ("v3", "v4"):
        u = lower(spec, ver=ver)
        shas[ver] = DveOpSpec(
            name=name, opcode=row, uops=u, rd1_en=has_src1(spec)
        ).sha(ver)
    op = dmod.DveOp(name, spec, subdim=False, uops_sha=shas)
    dmod.OPS.append(op)
    dmod.CUSTOM_DVE_SPECS[name] = spec
    dmod._SUB_OPCODE_FOR_NAME[name] = row
    return op


def _build_program():
    import concourse.bass as bass  # noqa: F401
    import concourse.mybir as mybir
    from concourse import bacc
    from concourse.masks import make_identity
    from concourse.tile import TileContext

    f32 = mybir.dt.float32
    bf16 = mybir.dt.bfloat16
    i32 = mybir.dt.int32

    expop = _register_exp_op()

    nc = bacc.Bacc(trn_type="TRN2")

    # [p, ht, j]: partition line = HT*S*2B = 32 KB contiguous
    valueT = nc.dram_tensor("valueT", [P, HT, S], bf16, kind="ExternalInput")
    # [p, ht, col]: partition line = HT*512*2B = 8 KB contiguous
    wv = nc.dram_tensor("wv", [P, HT, HPC * D], bf16, kind="ExternalInput")
    bvs = nc.dram_tensor("bvs", [1, HPC * D], bf16, kind="ExternalInput")
    # meta: per-partition [sb | eb-ln(A0) | ss | bb], each JT*HPC wide
    meta = nc.dram_tensor("meta", [P, 4 * JT * HPC], f32, kind="ExternalInput")
    # x broadcast to all partitions, fp16 (2 elem/cycle on ScalarE)
    xh = nc.dram_tensor("xh", [P, S], f32, kind="ExternalInput")
    # [hl, p, icq, d]: row i = icq*128 + p; partition line = JT*D*4B = 4 KB
    out = nc.dram_tensor("out", [HPC, P, JT, D], f32, kind="ExternalOutput")

    with TileContext(nc) as tc:
        with (
            tc.tile_pool(name="const", bufs=1) as const_pool,
            tc.tile_pool(name="vp", bufs=1) as vp_pool,
            tc.tile_pool(name="tt", bufs=TBUFS) as t_pool,
            tc.tile_pool(name="zz", bufs=3) as z_pool,
            tc.tile_pool(name="ps", bufs=2, space="PSUM") as ps_pool,
            tc.tile_pool(name="av", bufs=4, space="PSUM") as av_pool,
            tc.tile_pool(name="tp", bufs=2, space="PSUM") as tp_pool,
            tc.tile_pool(name="sp", bufs=4) as s_pool,
            tc.tile_pool(name="cp", bufs=3) as c_pool,
            tc.tile_pool(name="rp", bufs=3) as r_pool,
            tc.tile_pool(name="op", bufs=2) as o_pool,
        ):
            ident = const_pool.tile([P, P], f32)
            make_identity(nc, ident)
            ones1 = const_pool.tile([1, P], bf16)
            nc.vector.memset(ones1[:, :], 1.0)
            # meta first: the ScalarE exp stream (the bottleneck engine)
            # starts as soon as it lands; valueT is only needed ~15us in.
            meta_sb = const_pool.tile([P, 4 * JT * HPC], f32)
            nc.sync.dma_start(meta_sb[:, :], meta[:, :])
            xh_sb = const_pool.tile([P, S], f32)
            nc.sync.dma_start(xh_sb[:, :], xh[:, :])
            bv_sb = const_pool.tile([1, HPC * D], bf16)
            nc.sync.dma_start(bv_sb[:, :], bvs[:, :])
            wv_sb = const_pool.tile([P, HT, HPC * D], bf16)
            nc.sync.dma_start(wv_sb[:, :, :], wv[:, :, :])
            vt_sb = const_pool.tile([P, HT, S], bf16)
            for jc in range(4):
                nc.sync.dma_start(
                    vt_sb[:, :, jc * (S // 4) : (jc + 1) * (S // 4)],
                    valueT[:, :, jc * (S // 4) : (jc + 1) * (S // 4)],
                )
            sb_sb = meta_sb[:, 0 : JT * HPC].rearrange(
                "p (jt h) -> p jt h", h=HPC
            )
            eb_sb = meta_sb[:, JT * HPC : 2 * JT * HPC].rearrange(
                "p (jt h) -> p jt h", h=HPC
            )
            ss_sb = meta_sb[:, 2 * JT * HPC : 3 * JT * HPC].rearrange(
                "p (jt h) -> p jt h", h=HPC
            )
            bb_sb = meta_sb[:, 3 * JT * HPC : 4 * JT * HPC].rearrange(
                "p (jt h) -> p jt h", h=HPC
            )
            b1c = const_pool.tile([P, 1], f32)
            nc.gpsimd.memset(b1c[:, :], B1F)
            b1sb = b1c.broadcast_to((P, S))

            # V-plus: per head, [j-part, jt, D+1]; column D preset to 1.0 so
            # the AV matmul also produces the softmax denominator (row D).
            vp = vp_pool.tile([P, HPC, JT, D + 1], bf16)
            nc.gpsimd.memset(vp[:, :, :, D : D + 1], 1.0)

            # Phase 1 VP is interleaved with head 0's phase 2 below, so
            # the PE starts consuming tte tiles immediately and the exp
            # engines never stall on t_pool backpressure.
            def emit_vp_jt(jt):
                ps = ps_pool.tile([P, HPC * D], f32, space="PSUM")
                for ht in range(HT):
                    nc.tensor.matmul(
                        ps,
                        vt_sb[:, ht, jt * P : (jt + 1) * P],
                        wv_sb[:, ht, :],
                        start=(ht == 0),
                        stop=False,
                    )
                nc.tensor.matmul(
                    ps, ones1[:, :], bv_sb[:, :], start=False, stop=True
                )
                nc.vector.tensor_copy(
                    vp[:, :, jt, 0:D],
                    ps.rearrange("p (h d) -> p h d", h=HPC),
                )

            def emit_tte(hl, jt):
                tte = t_pool.tile([P, S], bf16)
                off = jt in off_jts_for(hl)
                if not off:
                    nc.scalar.activation(
                        tte,
                        xh_sb,
                        mybir.ActivationFunctionType.Exp,
                        bias=eb_sb[:, jt, hl : hl + 1],
                        scale=sb_sb[:, jt, hl : hl + 1],
                    )
                else:
                    z = z_pool.tile([P, S], i32)
                    nc.gpsimd.tensor_scalar(
                        z,
                        xh_sb,
                        ss_sb[:, jt, hl : hl + 1],
                        bb_sb[:, jt, hl : hl + 1],
                        mybir.AluOpType.mult,
                        mybir.AluOpType.add,
                    )
                    nc.vector._custom_dve(
                        expop,
                        out=tte,
                        in0=z.bitcast(f32),
                        in1=b1sb,
                        s0=MASKF,
                        s1=1.0,
                        imm2=B2F,
                    )
                return tte

            def alloc_avs():
                return [
                    av_pool.tile(
                        [D + 1, ICW], f32, name=f"av{ic}", tag="av", space="PSUM"
                    )
                    for ic in range(IC)
                ]

            def emit_av(avs, hl, jt, tte):
                for ic in range(IC):
                    nc.tensor.matmul(
                        avs[ic],
                        vp[:, hl, jt, :],
                        tte[:, ic * ICW : (ic + 1) * ICW],
                        start=(jt == 0),
                        stop=(jt == JT - 1),
                    )

            def emit_drain(hl, avs):
                otile = o_pool.tile([P, IC * IC, D], f32)
                stens = []
                for ic in range(IC):
                    sten = s_pool.tile([D + 1, ICW], f32)
                    nc.vector.tensor_copy(sten, avs[ic])
                    stens.append(sten)
                for ic in range(IC):
                    sten = stens[ic]
                    tp = tp_pool.tile([P, IC, D + 1], f32, space="PSUM")
                    for q in range(IC):
                        nc.tensor.transpose(
                            tp[:, q, :],
                            sten[:, q * P : (q + 1) * P],
                            ident[0 : D + 1, 0 : D + 1],
                        )
                    ctile = c_pool.tile([P, IC, D + 1], f32)
                    nc.vector.tensor_copy(ctile, tp)
                    rec = r_pool.tile([P, IC, 1], f32)
                    nc.vector.reciprocal(rec, ctile[:, :, D : D + 1])
                    nc.vector.tensor_tensor(
                        otile[:, ic * IC : (ic + 1) * IC, :],
                        ctile[:, :, 0:D],
                        rec.broadcast_to((P, IC, D)),
                        mybir.AluOpType.mult,
                    )
                    nc.sync.dma_start(
                        out[hl, :, ic * IC : (ic + 1) * IC, :],
                        otile[:, ic * IC : (ic + 1) * IC, :],
                    )

            # head 0 rides along with the V projection (one-jt software
            # pipeline so its AV matmuls never block the PE on the cast)
            avs0 = alloc_avs()
            emit_vp_jt(0)
            for jt in range(JT):
                if jt + 1 < JT:
                    emit_vp_jt(jt + 1)
                tte = emit_tte(0, jt)
                emit_av(avs0, 0, jt, tte)
            emit_drain(0, avs0)

            for hl in range(1, HPC):
                avs = alloc_avs()
                for jt in range(JT):
                    tte = emit_tte(hl, jt)
                    emit_av(avs, hl, jt, tte)
                emit_drain(hl, avs)
    nc.compile()  # bacc legalization: wait-splitting, reg alloc, nop fusion
    return nc


def _get_program():
    if "nc" not in _cache:
        _cache["nc"] = _build_program()
    return _cache["nc"]


def kernel(query, key, value, Wq, bq, Wk, bk, Wv, bv):
    import concourse.mybir as mybir
    from concourse import bass_utils

    bf16 = mybir.dt.np(mybir.dt.bfloat16)

    query = np.asarray(query, dtype=np.float32)
    key = np.asarray(key, dtype=np.float32)
    value = np.asarray(value, dtype=np.float32)
    Wq = np.asarray(Wq, dtype=np.float32)
    bq = np.asarray(bq, dtype=np.float32)
    Wk = np.asarray(Wk, dtype=np.float32)
    bk = np.asarray(bk, dtype=np.float32)
    Wv = np.asarray(Wv, dtype=np.float32)
    bv = np.asarray(bv, dtype=np.float32)

    wq2 = Wq.reshape(H, D)
    wk2 = Wk.reshape(H, D)
    bq2 = bq.reshape(H, D)
    A = np.einsum("hd,hd->h", wq2, wk2)  # Wq_h . Wk_h
    E = np.einsum("hd,hd->h", wk2, bq2)  # Wk_h . bq_h

    in_maps = []
    for c in range(N_CORES):
        b = c // 2
        g = c % 2
        heads = np.arange(g * HPC, (g + 1) * HPC)
        x = query[b, :, 0]  # (S,)
        y = key[b, :, 0]  # (S,)
        # scale[j, h] = A_h y_j / 8 ; bias[j, h] = E_h y_j / 8
        sc = (y[:, None] * (A[heads] / 8.0)[None, :]).astype(np.float32)
        bi = (y[:, None] * (E[heads] / 8.0)[None, :]).astype(np.float32)
        amax = np.abs(sc * np.abs(x).max() + np.abs(bi)).max()
        assert amax < 80.0, f"logit magnitude {amax} risks fp32 exp overflow"
        in_maps.append(
            {
                # [p, ht, j] = value[b][j, ht*128+p]
                "valueT": np.ascontiguousarray(
                    value[b].T.reshape(HT, P, S).transpose(1, 0, 2)
                ).astype(bf16),
                # [p, ht, col] = Wv[ht*128+p, g*512+col]
                "wv": np.ascontiguousarray(
                    Wv[:, g * HPC * D : (g + 1) * HPC * D]
                    .reshape(HT, P, HPC * D)
                    .transpose(1, 0, 2)
                ).astype(bf16),
                "bvs": bv[g * HPC * D : (g + 1) * HPC * D].reshape(1, HPC * D).astype(bf16),
                "meta": np.concatenate(
                    [
                        sc.reshape(JT, P, HPC).transpose(1, 0, 2).reshape(P, -1),
                        (bi - np.log(A0F))
                        .reshape(JT, P, HPC)
                        .transpose(1, 0, 2)
                        .reshape(P, -1),
                        (sc * LOG2E * EXPL)
                        .reshape(JT, P, HPC)
                        .transpose(1, 0, 2)
                        .reshape(P, -1),
                        (bi * LOG2E * EXPL + 127.0 * EXPL)
                        .reshape(JT, P, HPC)
                        .transpose(1, 0, 2)
                        .reshape(P, -1),
                    ],
                    axis=1,
                ).astype(np.float32),
                "xh": np.ascontiguousarray(
                    np.broadcast_to(x, (P, S))
                ).astype(np.float32),
            }
        )

    nc = _get_program()
    res = bass_utils.run_bass_kernel_spmd(
        nc, in_maps, core_ids=list(range(N_CORES))
    ).results

    full = np.empty((H * B, S, D), dtype=np.float32)
    for c in range(N_CORES):
        b = c // 2
        g = c % 2
        o = res[c]["out"]  # [HPC, P, JT, D]; row i = icq*128 + p
        for hl in range(HPC):
            full[(g * HPC + hl) * B + b] = (
                o[hl].transpose(1, 0, 2).reshape(S, D)
            )
    return full
